# revision 1
# baseline (speedup 1.0000x reference)
"""Trainium2 Bass kernel for AsymmetricQuantLinear — fp8 DoubleRow + rank-1 zero-point.

    x:             [4096, 4096]  f32
    weight_packed: [2048, 11008] int32 (two 4-bit nibbles per value)
    weight_scale:  [11008] f32
    weight_zero:   [11008] f32
    out = x @ ((unpack(weight_packed) - zero) * scale)   -> [4096, 11008] f32

Tensor-parallel over N across 8 NeuronCores (1376 cols each), x replicated.

Math: out = (x̂ @ q)·s − rowsum(x̂) ⊗ (z·s), with x̂ = x_hi + r on corrected
k-tiles. The nibble values q ∈ [0,15] are exact in fp8 e4m3, so the PE streams
RAW q tiles (no on-device dequant at all); an all-ones column appended to the
moving operand makes the PSUM accumulate rowsum(x̂) for free across the same
start/stop group (hi and residual passes included), and the flush applies the
exact rank-1 zero-point term plus the per-column scale in fp32 on the DVE.

The PE runs fp8 perf_mode=DoubleRow (2 k-planes per instruction, 2
MACs/cell/cycle, measured ~2x bf16). x is split x = x_hi + r (both e4m3); the
first CKP k-pairs also accumulate r@q, shrinking the x-quantization error from
2.96e-2 (CKP=0) to 1.81e-2 at (16+CKP)/16 fp8 passes.

Host prep is layout/precision only: transpose, nibble unpack, fp8/f32 casts,
pre-tiling so every device DMA is one contiguous run per partition.
"""

import numpy as np
import ml_dtypes

M, K, N = 4096, 4096, 11008
N_CORES = 8
N_SHARD = N // N_CORES          # 1376
P = 128
KT = K // P                     # 32 k-tiles
KP = KT // 2                    # 16 k-pairs (DoubleRow consumes 2 k-tiles)
MSW = 256                       # m columns fetched per x DMA (two 128-wide m-tiles)
MSUP = M // MSW                 # 16
NPAD = N_SHARD + 32             # 1408: pad keeps DoubleRow plane stride 32B-aligned
ONES_COL = N_SHARD              # col 1376 of each q tile holds 1.0 -> rowsum(x̂)
MM_CHUNKS = [(0, 512, 512), (512, 512, 512), (1024, 354, 352)]  # (n0, mm width, flush width)
CKP = 10                        # k-pairs with hi+lo residual correction (0..16)

F8 = ml_dtypes.float8_e4m3

_compiled = {}


def _build(ckp):
    import concourse.mybir as mybir
    import concourse.tile as tile
    from concourse import bacc

    f32 = mybir.dt.float32
    f8 = mybir.dt.float8e4
    DR = mybir.MatmulPerfMode.DoubleRow
    ALU = mybir.AluOpType

    nc = bacc.Bacc("TRN2", target_bir_lowering=False, debug=False, num_devices=N_CORES)
    xh = nc.dram_tensor("xh", [MSUP, P, KT, MSW], f8, kind="ExternalInput").ap()
    if ckp:
        xr = nc.dram_tensor("xr", [MSUP, P, 2 * ckp, MSW], f8, kind="ExternalInput").ap()
    q = nc.dram_tensor("q", [KP, P, 2, NPAD], f8, kind="ExternalInput").ap()
    s = nc.dram_tensor("s", [P, N_SHARD], f32, kind="ExternalInput").ap()
    zs = nc.dram_tensor("zs", [P, N_SHARD], f32, kind="ExternalInput").ap()  # -(zero*scale)
    out = nc.dram_tensor("out", [M, N_SHARD], f32, kind="ExternalOutput").ap()

    with tile.TileContext(nc) as tc:
        with (
            tc.tile_pool(name="const", bufs=1) as constp,
            tc.tile_pool(name="wq", bufs=1) as wqp,
            tc.tile_pool(name="xin", bufs=3) as xp,
            tc.tile_pool(name="xrin", bufs=3) as xrp,
            tc.tile_pool(name="ostage", bufs=3) as outp,
            tc.tile_pool(name="psum", space="PSUM", bufs=2) as pp,
        ):
            # W tiles are the raw q nibbles (exact in fp8) — DMA only, no
            # dequant. The x (and residual) transfers are woven into the q
            # stream in 4-ktile slices so supply tracks the first sweeps'
            # demand: x slices on even kp, xr on odd kp. Pair 0 is split into
            # per-chunk tiles (Tile tracks deps per tile, so the first matmul
            # then waits only on its own chunk's DMA), and the big s/zs
            # constant DMAs are deferred until after the first k-pairs — they
            # are not needed until the first flush.
            w_tiles = [None]
            CPAD = (512, 512, 384)

            w0_chunks = []
            for ci, (n0, nw, _) in enumerate(MM_CHUNKS):
                wc = wqp.tile([P, 2, CPAD[ci]], f8, tag=f"w0c{ci}", name="w0c")
                nc.sync.dma_start(wc[:, :, 0:nw], q[0, :, :, n0:n0 + nw])
                w0_chunks.append(wc)

            def w_slice(kp, ci, n0, nw):
                if kp == 0:
                    return w0_chunks[ci][:, :, 0:nw]
                return w_tiles[kp][:, :, n0:n0 + nw]

            def fetch_q(kp):
                wt = wqp.tile([P, 2, NPAD], f8, tag=f"w{kp}", name="wt")
                nc.sync.dma_start(wt[:], q[kp])
                w_tiles.append(wt)

            x0_t = xp.tile([P, KT, MSW], f8, tag="x", name="x_t")
            x1_t = xp.tile([P, KT, MSW], f8, tag="x", name="x_t")
            if ckp:
                xr0_t = xrp.tile([P, 2 * ckp, MSW], f8, tag="xr", name="xr_t")
                xr1_t = xrp.tile([P, 2 * ckp, MSW], f8, tag="xr", name="xr_t")
            nc.sync.dma_start(x0_t[:, 0:4, :], xh[0, :, 0:4, :])
            nc.sync.dma_start(x1_t[:, 0:4, :], xh[1, :, 0:4, :])
            fetch_q(1)
            if ckp:
                nc.sync.dma_start(xr0_t[:, 0:4, :], xr[0, :, 0:4, :])
                nc.sync.dma_start(xr1_t[:, 0:4, :], xr[1, :, 0:4, :])
            s_t = constp.tile([P, N_SHARD], f32, tag="s")
            nc.sync.dma_start(s_t[:], s[:])
            zs_t = constp.tile([P, N_SHARD], f32, tag="zs")
            nc.sync.dma_start(zs_t[:], zs[:])
            for kp in range(2, KP):
                fetch_q(kp)
                if kp % 2 == 0:
                    g = kp // 2
                    if 4 * g < KT:
                        nc.sync.dma_start(
                            x0_t[:, 4 * g:4 * g + 4, :], xh[0, :, 4 * g:4 * g + 4, :])
                        nc.sync.dma_start(
                            x1_t[:, 4 * g:4 * g + 4, :], xh[1, :, 4 * g:4 * g + 4, :])
                elif ckp:
                    g = (kp - 1) // 2
                    if 4 * g < 2 * ckp:
                        ge = min(4 * g + 4, 2 * ckp)
                        nc.sync.dma_start(
                            xr0_t[:, 4 * g:ge, :], xr[0, :, 4 * g:ge, :])
                        nc.sync.dma_start(
                            xr1_t[:, 4 * g:ge, :], xr[1, :, 4 * g:ge, :])

            def mm_sweep(ps_chunks, x_t, xr_t, sub):
                lhs = lambda t, kp: t[:, 2 * kp:2 * kp + 2, sub * P:(sub + 1) * P]
                for kp in range(KP):
                    for ci, (n0, nw, _) in enumerate(MM_CHUNKS):
                        nc.tensor.matmul(
                            ps_chunks[ci][:],
                            lhs(x_t, kp),
                            w_slice(kp, ci, n0, nw),
                            start=(kp == 0),
                            stop=(kp == KP - 1 and not ckp),
                            perf_mode=DR,
                        )
                for cp in range(ckp):
                    for ci, (n0, nw, _) in enumerate(MM_CHUNKS):
                        nc.tensor.matmul(
                            ps_chunks[ci][:],
                            lhs(xr_t, cp),
                            w_slice(cp, ci, n0, nw),
                            start=False,
                            stop=(cp == ckp - 1),
                            perf_mode=DR,
                        )

            def mm_sweep_interleaved(psss, x_t, xr_t):
                # Both m-subtiles interleaved in one k-sweep, and each
                # corrected pair's residual MMs issued right after its hi MMs:
                # every q-tile arrival unlocks up to 12 queued MMs.
                for kp in range(KP):
                    for sub in (0, 1):
                        lhsT = x_t[:, 2 * kp:2 * kp + 2, sub * P:(sub + 1) * P]
                        for ci, (n0, nw, _) in enumerate(MM_CHUNKS):
                            nc.tensor.matmul(
                                psss[sub][ci][:],
                                lhsT,
                                w_slice(kp, ci, n0, nw),
                                start=(kp == 0),
                                stop=(kp == KP - 1),
                                perf_mode=DR,
                            )
                    if kp < ckp:
                        for sub in (0, 1):
                            lhsT = xr_t[:, 2 * kp:2 * kp + 2, sub * P:(sub + 1) * P]
                            for ci, (n0, nw, _) in enumerate(MM_CHUNKS):
                                nc.tensor.matmul(
                                    psss[sub][ci][:],
                                    lhsT,
                                    w_slice(kp, ci, n0, nw),
                                    start=False,
                                    stop=False,
                                    perf_mode=DR,
                                )

            def flush(ps_chunks, o_t, msub):
                # PSUM -> SBUF: per-column scale, then the exact rank-1
                # zero-point term  o += rowsum(x̂) * (-(zero*scale)),
                # with rowsum(x̂) read from the ones-column of chunk 3.
                for ci, (n0, _, fw) in enumerate(MM_CHUNKS):
                    nc.vector.tensor_mul(
                        o_t[:, n0:n0 + fw], ps_chunks[ci][:, 0:fw], s_t[:, n0:n0 + fw])
                rs = ps_chunks[2][:, 352:353]
                nc.vector.scalar_tensor_tensor(
                    o_t[:], zs_t[:], rs, o_t[:], op0=ALU.mult, op1=ALU.add)
                m0 = msub * P
                nc.sync.dma_start(out[m0:m0 + P, :], o_t[:])

            for msi in range(MSUP):
                if msi == 0:
                    x_t, xr_t = x0_t, (xr0_t if ckp else None)
                elif msi == 1:
                    x_t, xr_t = x1_t, (xr1_t if ckp else None)
                else:
                    x_t = xp.tile([P, KT, MSW], f8, tag="x", name="x_t")
                    nc.sync.dma_start(x_t[:], xh[msi])
                    if ckp:
                        xr_t = xrp.tile([P, 2 * ckp, MSW], f8, tag="xr", name="xr_t")
                        nc.sync.dma_start(xr_t[:], xr[msi])
                    else:
                        xr_t = None
                if msi <= 1:
                    o_ts = [outp.tile([P, N_SHARD], f32, tag="o", name="o_t")
                            for _ in (0, 1)]
                    psss = [
                        [pp.tile([P, nw], f32, tag=f"ps{ci}", name=f"ps{ci}")
                         for ci, (n0, nw, _) in enumerate(MM_CHUNKS)]
                        for _ in (0, 1)
                    ]
                    mm_sweep_interleaved(psss, x_t, xr_t)
                    for sub in (0, 1):
                        flush(psss[sub], o_ts[sub], msi * 2 + sub)
                    continue
                for sub in (0, 1):
                    o_t = outp.tile([P, N_SHARD], f32, tag="o")
                    pss = [pp.tile([P, nw], f32, tag=f"ps{ci}", name=f"ps{ci}")
                           for ci, (n0, nw, _) in enumerate(MM_CHUNKS)]
                    mm_sweep(pss, x_t, xr_t, sub)
                    flush(pss, o_t, msi * 2 + sub)

    nc.compile()
    return nc


def _pretile(a, kt_n):
    # [kt_n*P, M] -> [MSUP, P, kt_n, MSW]; element [msi,p,kt,j] = a[kt*P+p, msi*MSW+j]
    return np.ascontiguousarray(a.reshape(kt_n, P, MSUP, MSW).transpose(2, 1, 0, 3))


def _prep_in_maps(x, weight_packed, weight_scale, weight_zero, ckp):
    x = np.asarray(x, dtype=np.float32)
    wp = np.asarray(weight_packed, dtype=np.int32)
    ws = np.asarray(weight_scale, dtype=np.float32)
    wz = np.asarray(weight_zero, dtype=np.float32)

    xt = np.ascontiguousarray(x.T)           # [K, M] f32
    xh8 = xt.astype(F8)                      # [K, M] fp8 hi part
    xh_tiled = _pretile(xh8, KT)
    if ckp:
        kc = 2 * ckp * P
        r8 = (xt[:kc] - xh8[:kc].astype(np.float32)).astype(F8)
        xr_tiled = _pretile(r8, 2 * ckp)

    qfull = np.empty((K, N), dtype=F8)
    qfull[0::2] = (wp & 15).astype(F8)
    qfull[1::2] = ((wp >> 4) & 15).astype(F8)
    zs_neg = (-wz * ws).astype(np.float32)

    in_maps = []
    for c in range(N_CORES):
        n0, n1 = c * N_SHARD, (c + 1) * N_SHARD
        # [KP, P, 2, NPAD]: nibbles, then a ones column (rowsum tap), zero pad.
        qc = np.zeros((KP, P, 2, NPAD), dtype=F8)
        qc[:, :, :, :N_SHARD] = (
            qfull[:, n0:n1].reshape(KP, 2, P, N_SHARD).transpose(0, 2, 1, 3))
        qc[:, :, :, ONES_COL] = F8(1.0)
        m = {
            "xh": xh_tiled,
            "q": qc,
            "s": np.ascontiguousarray(np.broadcast_to(ws[n0:n1][None, :], (P, N_SHARD))),
            "zs": np.ascontiguousarray(
                np.broadcast_to(zs_neg[n0:n1][None, :], (P, N_SHARD))),
        }
        if ckp:
            m["xr"] = xr_tiled
        in_maps.append(m)
    return in_maps


def run(x, weight_packed, weight_scale, weight_zero, trace=False, ckp=CKP, **spmd_kwargs):
    import time

    from concourse.bass_utils import run_bass_kernel_spmd

    if ckp not in _compiled:
        _compiled[ckp] = _build(ckp)
    in_maps = _prep_in_maps(x, weight_packed, weight_scale, weight_zero, ckp)
    last_err = None
    for attempt in range(3):
        try:
            res = run_bass_kernel_spmd(
                _compiled[ckp], in_maps, core_ids=list(range(N_CORES)), trace=trace,
                **spmd_kwargs,
            )
            break
        except Exception as e:  # transient wedged-device faults recover on retry
            last_err = e
            time.sleep(5)
    else:
        raise last_err
    full = np.concatenate([res.results[c]["out"] for c in range(N_CORES)], axis=1)
    return full, res


def kernel(x, weight_packed, weight_scale, weight_zero):
    full, _ = run(x, weight_packed, weight_scale, weight_zero, trace=False)
    return full

